# revision 8
# baseline (speedup 1.0000x reference)
"""Trainium2 Bass kernel for nn_MaxMarginLoss (segment_reduce).

Data-parallel over the batch: 32 samples -> 8 NeuronCores x 4 samples.

Per core, for each sample b:
  - segment sums over T=2048 timesteps into S=32 step buckets are computed
    on TensorE as mask[128t,32s].T @ |x|[128t,1024d], accumulated over 16
    K-chunks into PSUM (this is the memory-bound part: 32 MiB of `inputs`
    per core, streamed as 2 MiB contiguous DMAs).
  - the appearance-order logic avoids any sort: first-appearance positions
    come from a masked min-reduce; each step's rank is the count of
    strictly-smaller packed keys (pos*33 + id); the ordered-adjacency
    matrix A[i,j] = (rank_j == rank_i + 1 and j present) turns "gather by
    argsort and diff neighbours" into a tiny 32x32 matmul H_next = A @ H.
  - pair energies E_i = mean_d relu(H_i - H_next_i)^2 via Relu + Square
    with fused free-dim accumulation.
Each core returns [4,5] per-sample sums (npairs, n, ninv, sum E*valid,
sum relu(1-E)*inv); the host applies the binary labels and the final
scalar division (a few hundred flops).
"""

import numpy as np

import concourse.bass as bass
from concourse import mybir
from concourse.bass_utils import run_bass_kernel_spmd
from concourse.tile import TileContext

F32 = mybir.dt.float32
OP = mybir.AluOpType
AF = mybir.ActivationFunctionType

B, T, D = 32, 2048, 1024
S = 32          # step ids 1..32; id 0 is padding
ALPHA = 1.0
N_CORES = 8
BL = B // N_CORES           # samples per core
K = 128                     # matmul contraction tile (partitions)
NCHUNK = T // K             # 16 K-chunks per sample
XT = 4                      # K-chunks per x DMA ([128, XT, 1024] = 2 MiB)

# The public neuronxcc walrus (setupSyncWait in CoreV2/V3GenImpl) only
# supports a small number of embedded semaphore waits per instruction,
# while Tile's scheduler attaches one wait per required logical proc.
# After scheduling, hoist overflow waits onto same-engine no-ops placed
# immediately before the owning instruction: engine program order makes
# that semantically identical.
_MAX_WAITS_DEFAULT = 1
_MAX_WAITS_BY_OPCODE = {}


def _split_sync_waits(nc: bass.Bass):
    for f in nc.m.functions:
        for bb in f.blocks:
            insts = list(bb.instructions)
            need = []  # (ins, overflow_waits)
            for ins in insts:
                si = getattr(ins, "sync_info", None)
                if si is None or not si.on_wait:
                    continue
                cap = _MAX_WAITS_BY_OPCODE.get(ins.opcode, _MAX_WAITS_DEFAULT)
                waits = list(si.on_wait)
                if len(waits) <= cap:
                    continue
                ins.sync_info = mybir.SyncInfo(
                    on_wait=waits[:cap], on_update=list(si.on_update)
                )
                need.append((ins, waits[cap:], cap))
            if not need:
                continue
            nop_for: dict[str, list] = {}
            for ins, overflow, cap in need:
                eng = nc.engines[ins.engine]
                nops = []
                for i in range(0, len(overflow), cap):
                    nop = eng.nop(hint="waitsplit", nofuse=True)
                    nop.ins.sync_info = mybir.SyncInfo(
                        on_wait=overflow[i:i + cap], on_update=[]
                    )
                    nops.append(nop.ins)
                nop_for[ins.name] = nops
            created = {n.name for nops in nop_for.values() for n in nops}
            # nop() appended the new instructions to the current bb; pull
            # them out of every block and splice before their owners.
            for bb2 in f.blocks:
                cur = [i for i in bb2.instructions if i.name not in created]
                out = []
                for ins in cur:
                    out.extend(nop_for.get(ins.name, ()))
                    out.append(ins)
                bb2.instructions = out


def build_program() -> bass.Bass:
    nc = bass.Bass()

    x = nc.declare_dram_parameter("x", [BL, T, D], F32, isOutput=False)
    ids_bc = nc.declare_dram_parameter("ids_bc", [K, T], F32, isOutput=False)
    ids_ct = nc.declare_dram_parameter("ids_ct", [K, BL * NCHUNK], F32, isOutput=False)
    tmt = nc.declare_dram_parameter("tmt", [K, T], F32, isOutput=False)
    steps_col = nc.declare_dram_parameter("steps_col", [K, 1], F32, isOutput=False)
    iota_rows = nc.declare_dram_parameter("iota_rows", [K, S], F32, isOutput=False)
    lower = nc.declare_dram_parameter("lower", [K, S], F32, isOutput=False)
    ones_sq = nc.declare_dram_parameter("ones_sq", [K, S], F32, isOutput=False)
    blockones = nc.declare_dram_parameter("blockones", [K, BL], F32, isOutput=False)
    out5 = nc.declare_dram_parameter("out5", [BL, 5], F32, isOutput=True)

    with TileContext(nc) as tc:
        with (
            tc.tile_pool(name="const", bufs=1) as cpool,
            tc.tile_pool(name="persist", bufs=1) as pp,
            tc.tile_pool(name="xin", bufs=3) as xin,
            tc.tile_pool(name="xabs", bufs=3) as xabs,
            tc.tile_pool(name="mk", bufs=6) as mkp,
            tc.tile_pool(name="ps_sums", bufs=1, space="PSUM") as ps_sums,
            tc.tile_pool(name="ps_misc", bufs=1, space="PSUM") as ps_misc,
        ):
            # ---- constants / index data -------------------------------
            sb_idsbc = cpool.tile([K, T], F32)
            nc.sync.dma_start(out=sb_idsbc[:], in_=ids_bc[:])
            sb_idsct = cpool.tile([K, BL * NCHUNK], F32)
            nc.sync.dma_start(out=sb_idsct[:], in_=ids_ct[:])
            sb_tmt = cpool.tile([K, T], F32)
            nc.sync.dma_start(out=sb_tmt[:], in_=tmt[:])
            sb_steps = cpool.tile([K, 1], F32)
            nc.sync.dma_start(out=sb_steps[:], in_=steps_col[:])
            sb_iota = cpool.tile([K, S], F32)
            nc.sync.dma_start(out=sb_iota[:], in_=iota_rows[:])
            sb_lower = cpool.tile([K, S], F32)
            nc.sync.dma_start(out=sb_lower[:], in_=lower[:])
            sb_ones = cpool.tile([K, S], F32)
            nc.sync.dma_start(out=sb_ones[:], in_=ones_sq[:])
            sb_bones = cpool.tile([K, BL], F32)
            nc.sync.dma_start(out=sb_bones[:], in_=blockones[:])

            # ---- phase A: masks / positions / ranks (all 4 samples
            #      stacked on partitions: row 32*b + s) ------------------
            maskf = pp.tile([K, T], F32)        # [s-stacked, t] 0/1 mask
            counts = pp.tile([K, 1], F32)
            nc.vector.tensor_scalar(
                maskf[:], sb_idsbc[:], sb_steps[:], None, OP.is_equal, OP.add,
                accum_out=counts[:],
            )
            tm = pp.tile([K, T], F32)           # mask * (t - T)
            nc.vector.tensor_tensor(tm[:], maskf[:], sb_tmt[:], OP.mult)
            posm = pp.tile([K, 1], F32)         # pos - T (present) else 0
            nc.vector.tensor_reduce(posm[:], tm[:], mybir.AxisListType.X, OP.min)

            cnt1 = pp.tile([K, 1], F32)
            nc.vector.tensor_scalar(cnt1[:], counts[:], 1.0, None, OP.max)
            recip = pp.tile([K, 1], F32)        # 1 / max(counts, 1)
            nc.vector.reciprocal(recip[:], cnt1[:])

            # distinct sort keys: (pos-T)*33 + (s+1); order == stable
            # argsort of pos with id tiebreak (present strictly first)
            key = pp.tile([K, 1], F32)
            nc.vector.tensor_scalar(
                key[:], posm[:], 33.0, sb_steps[:], OP.mult, OP.add
            )
            key_sq = pp.tile([K, S], F32)
            nc.vector.tensor_scalar(key_sq[:], sb_ones[:], key[:], None, OP.mult)
            key_t = pp.tile([K, S], F32)        # row i holds key_l along l
            nc.vector.transpose(key_t[:], key_sq[:])
            cmp = pp.tile([K, S], F32)
            rank = pp.tile([K, 1], F32)
            nc.vector.tensor_scalar(
                cmp[:], key_t[:], key[:], None, OP.is_lt, OP.add,
                accum_out=rank[:],
            )
            rankp1 = pp.tile([K, 1], F32)
            nc.vector.tensor_scalar(rankp1[:], rank[:], 1.0, None, OP.add)
            t999 = pp.tile([K, 1], F32)         # 999 for absent steps
            nc.vector.tensor_scalar(
                t999[:], posm[:], 0.0, 999.0, OP.is_ge, OP.mult
            )
            rankp = pp.tile([K, 1], F32)        # rank, pushed out if absent
            nc.vector.tensor_tensor(rankp[:], rank[:], t999[:], OP.add)

            v_t = pp.tile([K, 8], F32)          # per-step stats columns
            nc.vector.tensor_scalar(v_t[:, 1:2], posm[:], 0.0, None, OP.is_lt)

            rankp_sq = pp.tile([K, S], F32)
            nc.vector.tensor_scalar(rankp_sq[:], sb_ones[:], rankp[:], None, OP.mult)
            rankp_t = pp.tile([K, S], F32)
            nc.vector.transpose(rankp_t[:], rankp_sq[:])
            rankp1_sq = pp.tile([K, S], F32)
            nc.vector.tensor_scalar(rankp1_sq[:], sb_ones[:], rankp1[:], None, OP.mult)
            rankp1_t = pp.tile([K, S], F32)
            nc.vector.transpose(rankp1_t[:], rankp1_sq[:])

            # A[i,j] = (rankp_j == rank_i + 1); succ_i = sum_j A[i,j]
            a_m = pp.tile([K, S], F32)
            nc.vector.tensor_scalar(
                a_m[:], rankp_t[:], rankp1[:], None, OP.is_equal, OP.add,
                accum_out=v_t[:, 0:1],
            )
            # A^T (lhsT for the H_next matmul)
            a_t = pp.tile([K, S], F32)
            nc.vector.tensor_scalar(
                a_t[:], rankp1_t[:], rankp[:], None, OP.is_equal
            )
            # inv_i = sum_j A[i,j] * [i > j]
            a_inv = pp.tile([K, S], F32)
            nc.vector.scalar_tensor_tensor(
                a_inv[:], rankp_t[:], rankp1[:], sb_lower[:],
                op0=OP.is_equal, op1=OP.mult, accum_out=v_t[:, 2:3],
            )

            # ---- phase B: segment sums via TensorE --------------------
            h_all = pp.tile([K, D], F32)
            ps_all = ps_sums.tile([K, D], F32)  # 2 PSUM banks
            for b in range(BL):
                for tq in range(NCHUNK // XT):
                    xt = xin.tile([K, XT, D], F32)
                    nc.sync.dma_start(
                        out=xt[:],
                        in_=x[b, tq * XT * K:(tq + 1) * XT * K, :].rearrange(
                            "(s p) d -> p s d", p=K
                        ),
                    )
                    xa = xabs.tile([K, XT, D], F32)
                    nc.scalar.activation(xa[:], xt[:], AF.Abs)
                    for sub in range(XT):
                        c = tq * XT + sub
                        mk = mkp.tile([K, S], F32)
                        nc.vector.tensor_scalar(
                            mk[:], sb_iota[:],
                            sb_idsct[:, b * NCHUNK + c:b * NCHUNK + c + 1],
                            None, OP.is_equal,
                        )
                        for h in range(2):
                            nc.tensor.matmul(
                                ps_all[b * S:(b + 1) * S, h * 512:(h + 1) * 512],
                                lhsT=mk[:],
                                rhs=xa[:, sub, h * 512:(h + 1) * 512],
                                start=(c == 0), stop=(c == NCHUNK - 1),
                                tile_position=(0, b * S),
                            )
            # H = sums / max(counts, 1)
            for h in range(2):
                nc.vector.tensor_scalar(
                    h_all[:, h * 512:(h + 1) * 512],
                    ps_all[:, h * 512:(h + 1) * 512],
                    recip[:], None, OP.mult,
                )

            # ---- phase C: H_next = A @ H, pair energies, reduction ----
            hn = ps_misc.tile([K, D], F32)      # 2 PSUM banks
            for b in range(BL):
                for h in range(2):
                    nc.tensor.matmul(
                        hn[b * S:(b + 1) * S, h * 512:(h + 1) * 512],
                        lhsT=a_t[b * S:(b + 1) * S, :],
                        rhs=h_all[b * S:(b + 1) * S, h * 512:(h + 1) * 512],
                        start=True, stop=True,
                        tile_position=(b * S, b * S),
                    )
            diff = pp.tile([K, D], F32)
            nc.vector.tensor_tensor(diff[:], h_all[:], hn[:], OP.subtract)
            rd = pp.tile([K, D], F32)
            nc.vector.tensor_scalar(rd[:], diff[:], 0.0, None, OP.max)
            sq = pp.tile([K, D], F32)
            e_raw = pp.tile([K, 1], F32)
            nc.scalar.activation(sq[:], rd[:], AF.Square, accum_out=e_raw[:])
            e_col = pp.tile([K, 1], F32)
            nc.vector.tensor_scalar(e_col[:], e_raw[:], 1.0 / D, None, OP.mult)
            nc.vector.tensor_tensor(v_t[:, 3:4], e_col[:], v_t[:, 0:1], OP.mult)
            ae1 = pp.tile([K, 1], F32)          # relu(ALPHA - E)
            nc.vector.tensor_scalar(
                ae1[:], e_col[:], -1.0, ALPHA, OP.mult, OP.add
            )
            ae = pp.tile([K, 1], F32)
            nc.vector.tensor_scalar(ae[:], ae1[:], 0.0, None, OP.max)
            nc.vector.tensor_tensor(v_t[:, 4:5], ae[:], v_t[:, 2:3], OP.mult)

            # per-sample column sums: blockones[128,4].T @ V[128,5] -> [4,5]
            vp = ps_misc.tile([BL, 8], F32)
            nc.tensor.matmul(
                vp[:, 0:5], lhsT=sb_bones[:], rhs=v_t[:, 0:5],
                start=True, stop=True,
            )
            out_sb = pp.tile([BL, 5], F32)
            nc.vector.tensor_copy(out_sb[:], vp[:, 0:5])
            nc.sync.dma_start(out=out5[:], in_=out_sb[:])

    _split_sync_waits(nc)
    return nc


_PROGRAM: bass.Bass | None = None


def get_program() -> bass.Bass:
    global _PROGRAM
    if _PROGRAM is None:
        _PROGRAM = build_program()
    return _PROGRAM


def make_in_maps(inputs: np.ndarray, step_ids: np.ndarray) -> list[dict]:
    """Shard + pre-layout the (tiny) index tensors per core."""
    inputs = np.ascontiguousarray(np.asarray(inputs, dtype=np.float32))
    step_ids = np.asarray(step_ids)

    steps_col = np.tile(np.arange(1, S + 1, dtype=np.float32), BL)[:, None]
    tmt = np.tile(
        (np.arange(T, dtype=np.float32) - T)[None, :], (K, 1)
    ).astype(np.float32)
    iota_rows = np.tile(np.arange(1, S + 1, dtype=np.float32)[None, :], (K, 1))
    lower = np.tile(
        (np.arange(S)[:, None] > np.arange(S)[None, :]).astype(np.float32),
        (BL, 1),
    )
    ones_sq = np.ones((K, S), dtype=np.float32)
    blockones = (
        (np.arange(K)[:, None] // S) == np.arange(BL)[None, :]
    ).astype(np.float32)

    in_maps = []
    for core in range(N_CORES):
        b0 = core * BL
        ids = step_ids[b0:b0 + BL].astype(np.float32)          # [4, 2048]
        ids_bc = np.repeat(ids, S, axis=0)                      # [128, 2048]
        # ids_ct[p, b*16 + c] = step_ids[b0+b, c*128 + p]
        ids_ct = np.ascontiguousarray(
            ids.reshape(BL, NCHUNK, K).transpose(2, 0, 1).reshape(K, BL * NCHUNK)
        )
        in_maps.append({
            "x": inputs[b0:b0 + BL],
            "ids_bc": np.ascontiguousarray(ids_bc),
            "ids_ct": ids_ct,
            "tmt": tmt,
            "steps_col": np.ascontiguousarray(steps_col),
            "iota_rows": np.ascontiguousarray(iota_rows),
            "lower": np.ascontiguousarray(lower),
            "ones_sq": ones_sq,
            "blockones": np.ascontiguousarray(blockones),
        })
    return in_maps


def finish_host(out5_per_core: list[np.ndarray], binary_labels: np.ndarray):
    """Combine per-sample (npairs, n, ninv, S1, S2) with labels."""
    v = np.concatenate([np.asarray(o, np.float64) for o in out5_per_core], axis=0)
    npairs, n, ninv, s1, s2 = v[:, 0], v[:, 1], v[:, 2], v[:, 3], v[:, 4]
    labels = np.asarray(binary_labels)
    loss_pos = s1 / np.maximum(npairs, 1.0)
    loss_neg = s2 / np.maximum(ninv, 1.0)
    pos_count = (labels == 1) & (n >= 2)
    neg_count = (labels == 0) & (ninv > 0)
    total = (loss_pos * pos_count).sum() + (loss_neg * neg_count).sum()
    num = pos_count.sum() + neg_count.sum()
    return np.float32(total / (num + 1e-9))


def kernel(inputs, step_ids, binary_labels, _trace=False):
    nc = get_program()
    in_maps = make_in_maps(inputs, step_ids)
    res = run_bass_kernel_spmd(
        nc, in_maps, core_ids=list(range(N_CORES)), trace=_trace
    )
    out = finish_host([r["out5"] for r in res.results], binary_labels)
    if _trace:
        return out, res
    return out


# revision 10
# speedup vs baseline: 1.1044x; 1.1044x over previous
"""Trainium2 Bass kernel for nn_MaxMarginLoss (segment_reduce).

Data-parallel over the batch: 32 samples -> 8 NeuronCores x 4 samples.

Per core, for each sample b:
  - segment sums over T=2048 timesteps into S=32 step buckets are computed
    on TensorE as mask[128t,32s].T @ |x|[128t,1024d], accumulated over 16
    K-chunks into PSUM (this is the memory-bound part: 32 MiB of `inputs`
    per core, streamed as 2 MiB contiguous DMAs).
  - the appearance-order logic avoids any sort: first-appearance positions
    come from a masked min-reduce; each step's rank is the count of
    strictly-smaller packed keys (pos*33 + id); the ordered-adjacency
    matrix A[i,j] = (rank_j == rank_i + 1 and j present) turns "gather by
    argsort and diff neighbours" into a tiny 32x32 matmul H_next = A @ H.
  - pair energies E_i = mean_d relu(H_i - H_next_i)^2 via Relu + Square
    with fused free-dim accumulation.
Each core returns [4,5] per-sample sums (npairs, n, ninv, sum E*valid,
sum relu(1-E)*inv); the host applies the binary labels and the final
scalar division (a few hundred flops).
"""

import numpy as np

import concourse.bass as bass
from concourse import mybir
from concourse.bass_utils import run_bass_kernel_spmd
from concourse.tile import TileContext

F32 = mybir.dt.float32
BF16 = mybir.dt.bfloat16
OP = mybir.AluOpType
AF = mybir.ActivationFunctionType

B, T, D = 32, 2048, 1024
S = 32          # step ids 1..32; id 0 is padding
ALPHA = 1.0
N_CORES = 8
BL = B // N_CORES           # samples per core
K = 128                     # matmul contraction tile (partitions)
NCHUNK = T // K             # 16 K-chunks per sample
XT = 4                      # K-chunks per x DMA ([128, XT, 1024] = 2 MiB)

# The public neuronxcc walrus (setupSyncWait in CoreV2/V3GenImpl) only
# supports a small number of embedded semaphore waits per instruction,
# while Tile's scheduler attaches one wait per required logical proc.
# After scheduling, hoist overflow waits onto same-engine no-ops placed
# immediately before the owning instruction: engine program order makes
# that semantically identical.
_MAX_WAITS_DEFAULT = 1
_MAX_WAITS_BY_OPCODE = {}


def _split_sync_waits(nc: bass.Bass):
    for f in nc.m.functions:
        for bb in f.blocks:
            insts = list(bb.instructions)
            need = []  # (ins, overflow_waits)
            for ins in insts:
                si = getattr(ins, "sync_info", None)
                if si is None or not si.on_wait:
                    continue
                cap = _MAX_WAITS_BY_OPCODE.get(ins.opcode, _MAX_WAITS_DEFAULT)
                waits = list(si.on_wait)
                if len(waits) <= cap:
                    continue
                ins.sync_info = mybir.SyncInfo(
                    on_wait=waits[:cap], on_update=list(si.on_update)
                )
                need.append((ins, waits[cap:], cap))
            if not need:
                continue
            nop_for: dict[str, list] = {}
            for ins, overflow, cap in need:
                eng = nc.engines[ins.engine]
                nops = []
                for i in range(0, len(overflow), cap):
                    nop = eng.nop(hint="waitsplit", nofuse=True)
                    nop.ins.sync_info = mybir.SyncInfo(
                        on_wait=overflow[i:i + cap], on_update=[]
                    )
                    nops.append(nop.ins)
                nop_for[ins.name] = nops
            created = {n.name for nops in nop_for.values() for n in nops}
            # nop() appended the new instructions to the current bb; pull
            # them out of every block and splice before their owners.
            for bb2 in f.blocks:
                cur = [i for i in bb2.instructions if i.name not in created]
                out = []
                for ins in cur:
                    out.extend(nop_for.get(ins.name, ()))
                    out.append(ins)
                bb2.instructions = out


def build_program() -> bass.Bass:
    nc = bass.Bass()

    x = nc.declare_dram_parameter("x", [BL, T, D], F32, isOutput=False)
    ids_bc = nc.declare_dram_parameter("ids_bc", [K, T], F32, isOutput=False)
    ids_ct = nc.declare_dram_parameter("ids_ct", [K, BL * NCHUNK], F32, isOutput=False)
    tmt = nc.declare_dram_parameter("tmt", [K, T], F32, isOutput=False)
    steps_col = nc.declare_dram_parameter("steps_col", [K, 1], F32, isOutput=False)
    iota_rows = nc.declare_dram_parameter("iota_rows", [K, S], F32, isOutput=False)
    lower = nc.declare_dram_parameter("lower", [K, S], F32, isOutput=False)
    ones_sq = nc.declare_dram_parameter("ones_sq", [K, S], F32, isOutput=False)
    blockones = nc.declare_dram_parameter("blockones", [K, BL], F32, isOutput=False)
    out5 = nc.declare_dram_parameter("out5", [BL, 5], F32, isOutput=True)

    with TileContext(nc) as tc:
        with (
            tc.tile_pool(name="const", bufs=1) as cpool,
            tc.tile_pool(name="persist", bufs=1) as pp,
            tc.tile_pool(name="xin", bufs=3) as xin,
            tc.tile_pool(name="xabs", bufs=3) as xabs,
            tc.tile_pool(name="mk", bufs=6) as mkp,
            tc.tile_pool(name="ps_sums", bufs=1, space="PSUM") as ps_sums,
            tc.tile_pool(name="ps_misc", bufs=1, space="PSUM") as ps_misc,
        ):
            # ---- constants / index data -------------------------------
            sb_idsbc = cpool.tile([K, T], F32)
            nc.sync.dma_start(out=sb_idsbc[:], in_=ids_bc[:])
            sb_idsct = cpool.tile([K, BL * NCHUNK], F32)
            nc.sync.dma_start(out=sb_idsct[:], in_=ids_ct[:])
            sb_tmt = cpool.tile([K, T], F32)
            nc.sync.dma_start(out=sb_tmt[:], in_=tmt[:])
            sb_steps = cpool.tile([K, 1], F32)
            nc.sync.dma_start(out=sb_steps[:], in_=steps_col[:])
            sb_iota = cpool.tile([K, S], F32)
            nc.sync.dma_start(out=sb_iota[:], in_=iota_rows[:])
            sb_lower = cpool.tile([K, S], F32)
            nc.sync.dma_start(out=sb_lower[:], in_=lower[:])
            sb_ones = cpool.tile([K, S], F32)
            nc.sync.dma_start(out=sb_ones[:], in_=ones_sq[:])
            sb_bones = cpool.tile([K, BL], F32)
            nc.sync.dma_start(out=sb_bones[:], in_=blockones[:])

            # ---- phase A: masks / positions / ranks (all 4 samples
            #      stacked on partitions: row 32*b + s) ------------------
            maskf = pp.tile([K, T], F32)        # [s-stacked, t] 0/1 mask
            counts = pp.tile([K, 1], F32)
            nc.vector.tensor_scalar(
                maskf[:], sb_idsbc[:], sb_steps[:], None, OP.is_equal, OP.add,
                accum_out=counts[:],
            )
            tm = pp.tile([K, T], F32)           # mask * (t - T)
            nc.vector.tensor_tensor(tm[:], maskf[:], sb_tmt[:], OP.mult)
            posm = pp.tile([K, 1], F32)         # pos - T (present) else 0
            nc.vector.tensor_reduce(posm[:], tm[:], mybir.AxisListType.X, OP.min)

            cnt1 = pp.tile([K, 1], F32)
            nc.vector.tensor_scalar(cnt1[:], counts[:], 1.0, None, OP.max)
            recip = pp.tile([K, 1], F32)        # 1 / max(counts, 1)
            nc.vector.reciprocal(recip[:], cnt1[:])

            # distinct sort keys: (pos-T)*33 + (s+1); order == stable
            # argsort of pos with id tiebreak (present strictly first)
            key = pp.tile([K, 1], F32)
            nc.vector.tensor_scalar(
                key[:], posm[:], 33.0, sb_steps[:], OP.mult, OP.add
            )
            key_sq = pp.tile([K, S], F32)
            nc.vector.tensor_scalar(key_sq[:], sb_ones[:], key[:], None, OP.mult)
            key_t = pp.tile([K, S], F32)        # row i holds key_l along l
            nc.vector.transpose(key_t[:], key_sq[:])
            cmp = pp.tile([K, S], F32)
            rank = pp.tile([K, 1], F32)
            nc.vector.tensor_scalar(
                cmp[:], key_t[:], key[:], None, OP.is_lt, OP.add,
                accum_out=rank[:],
            )
            rankp1 = pp.tile([K, 1], F32)
            nc.vector.tensor_scalar(rankp1[:], rank[:], 1.0, None, OP.add)
            t999 = pp.tile([K, 1], F32)         # 999 for absent steps
            nc.vector.tensor_scalar(
                t999[:], posm[:], 0.0, 999.0, OP.is_ge, OP.mult
            )
            rankp = pp.tile([K, 1], F32)        # rank, pushed out if absent
            nc.vector.tensor_tensor(rankp[:], rank[:], t999[:], OP.add)

            v_t = pp.tile([K, 8], F32)          # per-step stats columns
            nc.vector.tensor_scalar(v_t[:, 1:2], posm[:], 0.0, None, OP.is_lt)

            rankp_sq = pp.tile([K, S], F32)
            nc.vector.tensor_scalar(rankp_sq[:], sb_ones[:], rankp[:], None, OP.mult)
            rankp_t = pp.tile([K, S], F32)
            nc.vector.transpose(rankp_t[:], rankp_sq[:])
            rankp1_sq = pp.tile([K, S], F32)
            nc.vector.tensor_scalar(rankp1_sq[:], sb_ones[:], rankp1[:], None, OP.mult)
            rankp1_t = pp.tile([K, S], F32)
            nc.vector.transpose(rankp1_t[:], rankp1_sq[:])

            # A[i,j] = (rankp_j == rank_i + 1); succ_i = sum_j A[i,j]
            a_m = pp.tile([K, S], F32)
            nc.vector.tensor_scalar(
                a_m[:], rankp_t[:], rankp1[:], None, OP.is_equal, OP.add,
                accum_out=v_t[:, 0:1],
            )
            # A^T (lhsT for the H_next matmul)
            a_t = pp.tile([K, S], F32)
            nc.vector.tensor_scalar(
                a_t[:], rankp1_t[:], rankp[:], None, OP.is_equal
            )
            # inv_i = sum_j A[i,j] * [i > j]
            a_inv = pp.tile([K, S], F32)
            nc.vector.scalar_tensor_tensor(
                a_inv[:], rankp_t[:], rankp1[:], sb_lower[:],
                op0=OP.is_equal, op1=OP.mult, accum_out=v_t[:, 2:3],
            )

            # ---- phase B: segment sums via TensorE --------------------
            h_all = pp.tile([K, D], F32)
            ps_all = ps_sums.tile([K, D], F32)  # 2 PSUM banks
            for b in range(BL):
                for tq in range(NCHUNK // XT):
                    xt = xin.tile([K, XT, D], F32)
                    # alternate the two HWDGE rings (sync / scalar) so x
                    # streaming is not serialized on a single ring
                    dma_eng = nc.sync if (b * (NCHUNK // XT) + tq) % 2 == 0 else nc.scalar
                    dma_eng.dma_start(
                        out=xt[:],
                        in_=x[b, tq * XT * K:(tq + 1) * XT * K, :].rearrange(
                            "(s p) d -> p s d", p=K
                        ),
                    )
                    # |x| rounded to bf16: the PE runs bf16 at 1 cycle/row
                    # vs fp32's 4; the 2^-9 relative rounding on |x| washes
                    # out to ~1e-4 in the final loss (mask stays exact 0/1)
                    xa = xabs.tile([K, XT, D], BF16)
                    nc.scalar.activation(xa[:], xt[:], AF.Abs)
                    for sub in range(XT):
                        c = tq * XT + sub
                        mk = mkp.tile([K, S], BF16)
                        nc.vector.tensor_scalar(
                            mk[:], sb_iota[:],
                            sb_idsct[:, b * NCHUNK + c:b * NCHUNK + c + 1],
                            None, OP.is_equal,
                        )
                        for h in range(2):
                            nc.tensor.matmul(
                                ps_all[b * S:(b + 1) * S, h * 512:(h + 1) * 512],
                                lhsT=mk[:],
                                rhs=xa[:, sub, h * 512:(h + 1) * 512],
                                start=(c == 0), stop=(c == NCHUNK - 1),
                                tile_position=(0, b * S),
                            )
            # H = sums / max(counts, 1)
            for h in range(2):
                nc.vector.tensor_scalar(
                    h_all[:, h * 512:(h + 1) * 512],
                    ps_all[:, h * 512:(h + 1) * 512],
                    recip[:], None, OP.mult,
                )

            # ---- phase C: H_next = A @ H, pair energies, reduction ----
            hn = ps_misc.tile([K, D], F32)      # 2 PSUM banks
            for b in range(BL):
                for h in range(2):
                    nc.tensor.matmul(
                        hn[b * S:(b + 1) * S, h * 512:(h + 1) * 512],
                        lhsT=a_t[b * S:(b + 1) * S, :],
                        rhs=h_all[b * S:(b + 1) * S, h * 512:(h + 1) * 512],
                        start=True, stop=True,
                        tile_position=(b * S, b * S),
                    )
            diff = pp.tile([K, D], F32)
            nc.vector.tensor_tensor(diff[:], h_all[:], hn[:], OP.subtract)
            rd = pp.tile([K, D], F32)
            nc.vector.tensor_scalar(rd[:], diff[:], 0.0, None, OP.max)
            sq = pp.tile([K, D], F32)
            e_raw = pp.tile([K, 1], F32)
            nc.scalar.activation(sq[:], rd[:], AF.Square, accum_out=e_raw[:])
            e_col = pp.tile([K, 1], F32)
            nc.vector.tensor_scalar(e_col[:], e_raw[:], 1.0 / D, None, OP.mult)
            nc.vector.tensor_tensor(v_t[:, 3:4], e_col[:], v_t[:, 0:1], OP.mult)
            ae1 = pp.tile([K, 1], F32)          # relu(ALPHA - E)
            nc.vector.tensor_scalar(
                ae1[:], e_col[:], -1.0, ALPHA, OP.mult, OP.add
            )
            ae = pp.tile([K, 1], F32)
            nc.vector.tensor_scalar(ae[:], ae1[:], 0.0, None, OP.max)
            nc.vector.tensor_tensor(v_t[:, 4:5], ae[:], v_t[:, 2:3], OP.mult)

            # per-sample column sums: blockones[128,4].T @ V[128,5] -> [4,5]
            vp = ps_misc.tile([BL, 8], F32)
            nc.tensor.matmul(
                vp[:, 0:5], lhsT=sb_bones[:], rhs=v_t[:, 0:5],
                start=True, stop=True,
            )
            out_sb = pp.tile([BL, 5], F32)
            nc.vector.tensor_copy(out_sb[:], vp[:, 0:5])
            nc.sync.dma_start(out=out5[:], in_=out_sb[:])

    _split_sync_waits(nc)
    return nc


_PROGRAM: bass.Bass | None = None


def get_program() -> bass.Bass:
    global _PROGRAM
    if _PROGRAM is None:
        _PROGRAM = build_program()
    return _PROGRAM


def make_in_maps(inputs: np.ndarray, step_ids: np.ndarray) -> list[dict]:
    """Shard + pre-layout the (tiny) index tensors per core."""
    inputs = np.ascontiguousarray(np.asarray(inputs, dtype=np.float32))
    step_ids = np.asarray(step_ids)

    steps_col = np.tile(np.arange(1, S + 1, dtype=np.float32), BL)[:, None]
    tmt = np.tile(
        (np.arange(T, dtype=np.float32) - T)[None, :], (K, 1)
    ).astype(np.float32)
    iota_rows = np.tile(np.arange(1, S + 1, dtype=np.float32)[None, :], (K, 1))
    lower = np.tile(
        (np.arange(S)[:, None] > np.arange(S)[None, :]).astype(np.float32),
        (BL, 1),
    )
    ones_sq = np.ones((K, S), dtype=np.float32)
    blockones = (
        (np.arange(K)[:, None] // S) == np.arange(BL)[None, :]
    ).astype(np.float32)

    in_maps = []
    for core in range(N_CORES):
        b0 = core * BL
        ids = step_ids[b0:b0 + BL].astype(np.float32)          # [4, 2048]
        ids_bc = np.repeat(ids, S, axis=0)                      # [128, 2048]
        # ids_ct[p, b*16 + c] = step_ids[b0+b, c*128 + p]
        ids_ct = np.ascontiguousarray(
            ids.reshape(BL, NCHUNK, K).transpose(2, 0, 1).reshape(K, BL * NCHUNK)
        )
        in_maps.append({
            "x": inputs[b0:b0 + BL],
            "ids_bc": np.ascontiguousarray(ids_bc),
            "ids_ct": ids_ct,
            "tmt": tmt,
            "steps_col": np.ascontiguousarray(steps_col),
            "iota_rows": np.ascontiguousarray(iota_rows),
            "lower": np.ascontiguousarray(lower),
            "ones_sq": ones_sq,
            "blockones": np.ascontiguousarray(blockones),
        })
    return in_maps


def finish_host(out5_per_core: list[np.ndarray], binary_labels: np.ndarray):
    """Combine per-sample (npairs, n, ninv, S1, S2) with labels."""
    v = np.concatenate([np.asarray(o, np.float64) for o in out5_per_core], axis=0)
    npairs, n, ninv, s1, s2 = v[:, 0], v[:, 1], v[:, 2], v[:, 3], v[:, 4]
    labels = np.asarray(binary_labels)
    loss_pos = s1 / np.maximum(npairs, 1.0)
    loss_neg = s2 / np.maximum(ninv, 1.0)
    pos_count = (labels == 1) & (n >= 2)
    neg_count = (labels == 0) & (ninv > 0)
    total = (loss_pos * pos_count).sum() + (loss_neg * neg_count).sum()
    num = pos_count.sum() + neg_count.sum()
    return np.float32(total / (num + 1e-9))


def kernel(inputs, step_ids, binary_labels, _trace=False):
    nc = get_program()
    in_maps = make_in_maps(inputs, step_ids)
    res = run_bass_kernel_spmd(
        nc, in_maps, core_ids=list(range(N_CORES)), trace=_trace
    )
    out = finish_host([r["out5"] for r in res.results], binary_labels)
    if _trace:
        return out, res
    return out
